# revision 23
# baseline (speedup 1.0000x reference)
"""Trainium2 Bass kernel for nn_Attention_45183055954094.

Cosine-similarity attention (temp=30) over 64 independent instances of
1024 tokens x 128 channels, with shared QK projection to head dim 32,
residual, and InstanceL2Norm. Data-parallel over the 64 instances across
8 NeuronCores (8 instances per core).

v2 design notes (vs v1 baseline at ~224us):
  - Single ACT table set (natural_log_exp_and_others: exp/ln/square/
    identity) for the whole kernel: every rsqrt is exp(-0.5*ln(x)+c),
    so there are ZERO table switches and phases interleave freely,
    keeping the PE HAM-warm (v1 lost ~60us to K=4/8 clock gating).
  - S_T matmuls (K=32) spread across all four 32-row PE bands via
    explicit tile_position -> concurrent execution (~3x on this shape).
  - Z (softmax denominator) no longer uses 16 PE matmuls: exp writes E
    in bf16, a chained DVE add (2x/4x perf mode) folds the 8 E tiles
    into P, and ONE all-ones [128,128] bf16 stationary matmul pair
    produces Z broadcast to all 128 partitions (no row evacuation).
  - k-side L2-norm is compact: per-k 30/|K| lands as a [128,8] column
    tile used as the exp's per-partition scale AP.
  - q-side norm: ssq row via 1-col-stationary matmuls, ln/exp on the
    row, then a stride-0 DMA broadcasts 1/|Q| to all 128 partitions.
  - Values path in bf16: f2 cast once, PE transposes in bf16, AV
    matmuls bf16 x bf16.
  - f1/f2 feed the projections via .bitcast(float32r) (no DVE casts).
  - Final InstanceL2Norm scale applied on the Pool engine (gpsimd);
    epilogue ssq via DVE tensor_tensor_reduce.
  - Software pipeline: phase_a(i+2) is emitted inside phase_b(i) so the
    PE instruction stream never goes sparse.
"""

import math
import sys

for _p in ("/opt/trn_rl_repo", "/root/.axon_site/_ro/trn_rl_repo"):
    if _p not in sys.path:
        sys.path.insert(0, _p)

import numpy as np

B, N, C, H, W = 16, 4, 128, 32, 32
HW = H * W           # 1024 tokens
NI = B * N           # 64 instances
NCORES = 8
IPC = NI // NCORES   # 8 instances per core
GRP = 4              # instances per epilogue group

_CACHE = {}


def _patch_act_tables():
    """Prefer the one table set that serves every activation this kernel
    uses (square/ln/exp).  The table-load placement pass picks the first
    set containing each function; the default ordering makes ln and exp
    resolve to different sets, thrashing ACT_TABLE_LOAD (~2.7us per
    switch).  Stripping this kernel's functions from every other set
    (order and indices preserved) makes natural_log_exp_and_others the
    unique match, so one table set stays resident for the whole run."""
    from concourse import bacc as _bacc

    if getattr(_bacc, "_act_tables_reordered", False):
        return
    orig = _bacc.get_activation_tables

    def filtered(arch):
        tabs = dict(orig(arch))
        pref = "natural_log_exp_and_others"
        if pref not in tabs:
            return tabs
        mine = {f for f in tabs[pref]}
        out = {}
        for k, v in tabs.items():
            out[k] = v if k == pref else v - mine
        return out

    _bacc.get_activation_tables = filtered
    _bacc._act_tables_reordered = True


def _build(ipc=IPC, grp=GRP):
    import concourse.bass as bass
    import concourse.tile as tile
    from concourse import bacc, mybir
    from concourse.bass import ts

    _patch_act_tables()

    f32 = mybir.dt.float32
    f32r = mybir.dt.float32r
    bf16 = mybir.dt.bfloat16
    AF = mybir.ActivationFunctionType

    nc = bacc.Bacc("TRN2", target_bir_lowering=False, debug=False)

    f1_d = nc.dram_tensor("f1", [ipc, C, HW], f32, kind="ExternalInput").ap()
    f2_d = nc.dram_tensor("f2", [ipc, C, HW], f32, kind="ExternalInput").ap()
    wt_d = nc.dram_tensor("wt", [C, C], f32, kind="ExternalInput").ap()
    b1_d = nc.dram_tensor("b1", [C, 1], f32, kind="ExternalInput").ap()
    b2_d = nc.dram_tensor("b2", [C, 1], f32, kind="ExternalInput").ap()
    id_d = nc.dram_tensor("ident", [C, C], f32, kind="ExternalInput").ap()
    t2_d = nc.dram_tensor("t2x2", [C, 1], f32, kind="ExternalInput").ap()
    out_d = nc.dram_tensor("out", [ipc, C, HW], f32, kind="ExternalOutput").ap()

    LN2 = math.log(2.0)
    LN60 = math.log(60.0)
    LN8 = math.log(8.0)

    with tile.TileContext(nc) as tc:
        import contextlib

        with contextlib.ExitStack() as ctx:
            consts = ctx.enter_context(tc.tile_pool(name="consts", bufs=1))
            fin = ctx.enter_context(tc.tile_pool(name="fin", bufs=2))
            f2p = ctx.enter_context(tc.tile_pool(name="f2p", bufs=4))
            f2bfp = ctx.enter_context(tc.tile_pool(name="f2bfp", bufs=2))
            sqp = ctx.enter_context(tc.tile_pool(name="sqp", bufs=2))
            binvqp = ctx.enter_context(tc.tile_pool(name="binvqp", bufs=1))
            qtnp = ctx.enter_context(tc.tile_pool(name="qtnp", bufs=3))
            ktp = ctx.enter_context(tc.tile_pool(name="ktp", bufs=3))
            x2tp = ctx.enter_context(tc.tile_pool(name="x2tp", bufs=3))
            kcp = ctx.enter_context(tc.tile_pool(name="kcp", bufs=6))
            rowp = ctx.enter_context(tc.tile_pool(name="rowp", bufs=2))
            ep = ctx.enter_context(tc.tile_pool(name="ep", bufs=8))
            pp = ctx.enter_context(tc.tile_pool(name="pp", bufs=2))
            bzp = ctx.enter_context(tc.tile_pool(name="bzp", bufs=1))
            t1p = ctx.enter_context(tc.tile_pool(name="t1p", bufs=2))
            rp = ctx.enter_context(tc.tile_pool(name="rp", bufs=6))
            scrp = ctx.enter_context(tc.tile_pool(name="scrp", bufs=1))
            colp = ctx.enter_context(tc.tile_pool(name="colp", bufs=16))
            op = ctx.enter_context(tc.tile_pool(name="op", bufs=2))
            # PSUM: ps 3x4KB slots + ps2 2x2KB slots = 16KB/partition
            ps = ctx.enter_context(tc.tile_pool(name="ps", bufs=3, space="PSUM"))
            ps2 = ctx.enter_context(tc.tile_pool(name="ps2", bufs=2, space="PSUM"))

            # ---- constants ----
            wt_sb = consts.tile([C, C], f32, tag="wt")
            nc.sync.dma_start(wt_sb[:], wt_d[:])
            wt_bf = consts.tile([C, C], bf16, tag="wtbf")
            nc.vector.tensor_copy(wt_bf[:], wt_sb[:])
            b1_sb = consts.tile([C, 1], f32, tag="b1")
            nc.sync.dma_start(b1_sb[:], b1_d[:])
            b2_sb = consts.tile([C, 1], f32, tag="b2")
            nc.sync.dma_start(b2_sb[:], b2_d[:])
            t2_sb = consts.tile([C, 1], f32, tag="t2")
            nc.sync.dma_start(t2_sb[:], t2_d[:])
            id_sb = consts.tile([C, C], f32, tag="id")
            nc.sync.dma_start(id_sb[:], id_d[:])
            id_bf = consts.tile([C, C], bf16, tag="idbf")
            nc.vector.tensor_copy(id_bf[:], id_sb[:])

            ones128_f = consts.tile([C, 1], f32, tag="ones128f")
            nc.vector.memset(ones128_f[:], 1.0)
            ones128_r = consts.tile([C, 1], f32r, tag="ones128r")
            nc.vector.tensor_copy(ones128_r[:], ones128_f[:])

            onesrow_f = consts.tile([1, C], f32, tag="onesrowf")
            nc.vector.memset(onesrow_f[:], 1.0)
            onesrow_r = consts.tile([1, C], f32r, tag="onesrowr")
            nc.vector.tensor_copy(onesrow_r[:], onesrow_f[:])

            onessq_bf = consts.tile([C, C], bf16, tag="onessqbf")
            nc.vector.memset(onessq_bf[:], 1.0)

            ln2_sb = consts.tile([C, 1], f32, tag="ln2")
            nc.vector.memset(ln2_sb[:], LN2)
            ln60_sb = consts.tile([C, 1], f32, tag="ln60")
            nc.vector.memset(ln60_sb[:], LN60)
            ln8_sb = consts.tile([C, 1], f32, tag="ln8")
            nc.vector.memset(ln8_sb[:], LN8)
            eps_sb = consts.tile([C, 1], f32, tag="eps")
            nc.vector.memset(eps_sb[:], 1e-5)

            qtn_sbs = {}
            kt_sbs = {}
            x2t_sbs = {}
            f2_sbs = {}
            kinv_sbs = {}
            r_sbs = {}
            ssq_cols = {}

            def fr(ap):
                return ap.bitcast(f32r)

            def phase_a(i):
                """DMA + projections + norms + value transpose for inst i."""
                f1_sb = fin.tile([C, HW], f32, tag="fin")
                for h in range(2):
                    nc.sync.dma_start(f1_sb[:, ts(h, 512)],
                                      f1_d[i, :, ts(h, 512)])
                f2_sb = f2p.tile([C, HW], f32, tag="f2")
                for h in range(2):
                    nc.sync.dma_start(f2_sb[:, ts(h, 512)],
                                      f2_d[i, :, ts(h, 512)])
                f2_sbs[i] = f2_sb
                f2bf = f2bfp.tile([C, HW], bf16, tag="f2bf")
                f1bf = f2bfp.tile([C, HW], bf16, tag="f1bf")
                for h in range(2):
                    nc.vector.tensor_copy(f2bf[:, ts(h, 512)],
                                          f2_sb[:, ts(h, 512)])
                for h in range(2):
                    nc.vector.tensor_copy(f1bf[:, ts(h, 512)],
                                          f1_sb[:, ts(h, 512)])

                # ---- k side ----
                psum_k = ps.tile([C, HW], f32, tag="ps")
                for h in range(2):
                    nc.tensor.matmul(psum_k[:, ts(h, 512)], wt_bf[:],
                                     f2bf[:, ts(h, 512)],
                                     start=True, stop=True)
                sq_k = sqp.tile([C, HW], f32, tag="sqk")
                nc.scalar.activation(sq_k[:], psum_k[:], AF.Square,
                                     bias=b2_sb[:])
                kt = ktp.tile([C, HW], f32r, tag="kt")
                nc.vector.tensor_scalar_add(kt[:], psum_k[:], b2_sb[:])
                kt_sbs[i] = kt
                psum_kc = ps2.tile([C, 8], f32, tag="sm")
                for j in range(8):
                    nc.tensor.matmul(psum_kc[:, j:j + 1],
                                     sq_k[:, ts(j, C)], ones128_f[:],
                                     start=True, stop=True)
                lnkc = kcp.tile([C, 8], f32, tag="lnkc")
                nc.scalar.activation(lnkc[:], psum_kc[:], AF.Ln)
                binv30 = kcp.tile([C, 8], f32, tag="binv30")
                nc.scalar.activation(binv30[:], lnkc[:], AF.Exp,
                                     scale=-0.5, bias=ln60_sb[:])
                kinv_sbs[i] = binv30

                # ---- value transpose (bf16) ----
                psum_t = ps2.tile([C, HW], bf16, tag="sm")
                for j in range(8):
                    nc.tensor.transpose(psum_t[:, ts(j, C)],
                                        f2bf[:, ts(j, C)], id_bf[:])
                x2t = x2tp.tile([C, HW], bf16, tag="x2t")
                nc.vector.tensor_copy(x2t[:], psum_t[:])
                x2t_sbs[i] = x2t

                # ---- q side ----
                psum_q = ps.tile([C, HW], f32, tag="ps")
                for h in range(2):
                    nc.tensor.matmul(psum_q[:, ts(h, 512)], wt_bf[:],
                                     f1bf[:, ts(h, 512)],
                                     start=True, stop=True)
                sq_q = sqp.tile([C, HW], f32r, tag="sq")
                nc.scalar.activation(sq_q[:], psum_q[:], AF.Square,
                                     bias=b1_sb[:])
                lnq = rowp.tile([1, HW], f32r, tag="lnq")
                for h in range(2):
                    qr_h = ps2.tile([1, 512], f32, tag="sm")
                    nc.tensor.matmul(qr_h[0:1, :], ones128_r[:],
                                     sq_q[:, ts(h, 512)],
                                     start=True, stop=True)
                    nc.scalar.activation(lnq[0:1, ts(h, 512)], qr_h[0:1, :],
                                         AF.Ln)
                psum_lnb = ps.tile([C, HW], f32, tag="ps")
                for h in range(2):
                    nc.tensor.matmul(psum_lnb[:, ts(h, 512)], onesrow_r[:],
                                     lnq[0:1, ts(h, 512)],
                                     start=True, stop=True)
                binvq = binvqp.tile([C, HW], f32, tag="binvq")
                nc.scalar.activation(binvq[:], psum_lnb[:], AF.Exp,
                                     scale=-0.5, bias=ln2_sb[:])
                qtn = qtnp.tile([C, HW], f32r, tag="qtn")
                junk_col = colp.tile([C, 1], f32, tag="junk")
                nc.vector.affine_mul_reduce(out=qtn[:], accum_out=junk_col[:],
                                            in0=psum_q[:], in1=binvq[:],
                                            scale=1.0, bias=b1_sb[:])
                qtn_sbs[i] = qtn

            state = {}

            def st_mms(i, j):
                """S_T block j: K=32 matmuls on alternating PE row bands."""
                qtn, kt = qtn_sbs[i], kt_sbs[i]
                psum_s = ps.tile([C, HW], f32, tag="ps")
                for h in range(2):
                    rg = 32 * ((2 * j + h) % 4)
                    nc.tensor.matmul(psum_s[:, ts(h, 512)],
                                     kt[rg:rg + 32, ts(j, C)],
                                     qtn[rg:rg + 32, ts(h, 512)],
                                     start=True, stop=True,
                                     tile_position=(rg, 0))
                return psum_s

            def phase_b_loop(i, nxt, pending_tail, mid_emit=None):
                """S_T/exp pipeline + DVE partial-sum chain + AV block."""
                s_tiles = state.pop(("spre", i), {})
                if pending_tail is not None:
                    pending_tail()
                binv30 = kinv_sbs.pop(i)
                e_sbs = []
                p_acc = None
                for j in range(8):
                    if j not in s_tiles:
                        s_tiles[j] = st_mms(i, j)
                    e_sb = ep.tile([C, HW], bf16, tag="e")
                    nc.scalar.activation(e_sb[:], s_tiles.pop(j)[:], AF.Exp,
                                         scale=binv30[:, j:j + 1])
                    e_sbs.append(e_sb)
                    if j + 2 < 8:
                        s_tiles[j + 2] = st_mms(i, j + 2)
                    if j == 1:
                        p_acc = pp.tile([C, HW], bf16, tag="p")
                        nc.vector.tensor_add(p_acc[:], e_sbs[0][:],
                                             e_sbs[1][:])
                    elif j >= 2:
                        nc.vector.tensor_add(p_acc[:], p_acc[:], e_sb[:])
                # AV block
                x2t = x2t_sbs.pop(i)
                psum_av = ps.tile([C, HW], f32, tag="ps")
                for j in range(8):
                    for h in range(2):
                        sl = ts(h, 512)
                        nc.tensor.matmul(psum_av[:, sl],
                                         x2t[:, ts(j, C)],
                                         e_sbs[j][:, sl],
                                         start=(j == 0), stop=(j == 7))
                # Z broadcast to all 128 partitions via all-ones stationary
                psum_zb = ps.tile([C, HW], f32, tag="ps")
                for h in range(2):
                    nc.tensor.matmul(psum_zb[:, ts(h, 512)], onessq_bf[:],
                                     p_acc[:, ts(h, 512)],
                                     start=True, stop=True)
                bzr = bzp.tile([C, HW], f32, tag="bzr")
                nc.vector.reciprocal_approx_fast(bzr[:], psum_zb[:])
                t1 = t1p.tile([C, HW], f32, tag="t1")
                nc.vector.tensor_mul(t1[:], psum_av[:], bzr[:])
                state[i] = {"t1": t1}
                qtn_sbs.pop(i)
                kt_sbs.pop(i)
                if mid_emit is not None:
                    mid_emit()
                if nxt is not None:
                    state[("spre", nxt)] = {0: st_mms(nxt, 0)}

            def phase_b_tail(i):
                st = state.pop(i)
                t1 = st["t1"]
                r_sb = rp.tile([C, HW], f32, tag="r")
                nc.vector.affine_then_add(r_sb[:], f2_sbs.pop(i)[:], t1[:],
                                          scale=1.0, bias=t2_sb[:])
                r_sbs[i] = r_sb
                scr = scrp.tile([C, HW], f32, tag="scr")
                ssq_col = colp.tile([C, 1], f32, tag="ssqc")
                nc.scalar.activation(scr[:], r_sb[:], AF.Square,
                                     accum_out=ssq_col[:])
                ssq_cols[i] = ssq_col

            def phase_c_group(ids):
                """Batched InstanceL2Norm epilogue (ln/exp, Pool muls)."""
                ids = list(ids)
                n = len(ids)
                psum_g = ps2.tile([1, n], f32, tag="sm")
                for k, i in enumerate(ids):
                    nc.tensor.matmul(psum_g[:, k:k + 1], ones128_f[:],
                                     ssq_cols[i][:], start=True, stop=True)
                g_ln = colp.tile([1, n], f32, tag="gln")
                nc.scalar.activation(g_ln[:], psum_g[:], AF.Ln,
                                     scale=1.0, bias=eps_sb[0:1, :])
                g_sb = colp.tile([1, n], f32, tag="g")
                nc.scalar.activation(g_sb[:], g_ln[:], AF.Exp,
                                     scale=-0.5, bias=ln8_sb[0:1, :])
                psum_gc = ps2.tile([C, n], f32, tag="sm")
                for k in range(n):
                    nc.tensor.matmul(psum_gc[:, k:k + 1], onesrow_f[:],
                                     g_sb[:, k:k + 1], start=True, stop=True)
                g_cols = colp.tile([C, n], f32, tag="gc")
                nc.vector.tensor_copy(g_cols[:], psum_gc[:])
                for k, i in enumerate(ids):
                    o_sb = op.tile([C, HW], f32, tag="o")
                    for h in range(2):
                        sl = ts(h, 512)
                        nc.gpsimd.tensor_scalar_mul(o_sb[:, sl],
                                                    r_sbs[i][:, sl],
                                                    g_cols[:, k:k + 1])
                        nc.sync.dma_start(out_d[i, :, sl], o_sb[:, sl])
                for i in ids:
                    del r_sbs[i]

            # ---- software pipeline ----
            phase_a(0)
            phase_a(1)
            pending = None
            for i in range(ipc):
                nxt = i + 1 if i + 1 < ipc else None

                def mid(i=i):
                    if i + 2 < ipc:
                        phase_a(i + 2)
                    if i == ipc - 3:
                        phase_c_group(range(0, grp))

                phase_b_loop(i, nxt, pending, mid_emit=mid)
                pending = (lambda p: lambda: phase_b_tail(p))(i)
            if pending is not None:
                pending()
            phase_c_group(range(grp, ipc))

    nc.compile()
    return nc


def kernel(**inputs) -> np.ndarray:
    from concourse import bass_utils

    f_list1 = np.asarray(inputs["f_list1"], dtype=np.float32)
    f_list2 = np.asarray(inputs["f_list2"], dtype=np.float32)
    t_pos1 = np.asarray(inputs["t_pos1"], dtype=np.float32).reshape(C)
    t_pos2 = np.asarray(inputs["t_pos2"], dtype=np.float32).reshape(C)
    W_qk_w = np.asarray(inputs["W_qk_w"], dtype=np.float32)
    W_qk_b = np.asarray(inputs["W_qk_b"], dtype=np.float32)

    # fold t_pos into the projection biases: q = W @ (x + t1) + b
    b1v = (W_qk_w @ t_pos1 + W_qk_b).astype(np.float32).reshape(32, 1)
    b2v = (W_qk_w @ t_pos2 + W_qk_b).astype(np.float32).reshape(32, 1)
    b1 = np.tile(b1v, (4, 1))                           # (128, 1)
    b2 = np.tile(b2v, (4, 1))
    ident = np.eye(C, dtype=np.float32)
    wt = np.ascontiguousarray(np.tile(W_qk_w.T, (1, 4)))  # (128, 128)
    t2x2 = (2.0 * t_pos2).astype(np.float32).reshape(C, 1)

    f1 = np.ascontiguousarray(f_list1.reshape(NI, C, HW))
    f2 = np.ascontiguousarray(f_list2.reshape(NI, C, HW))

    if "nc" not in _CACHE:
        _CACHE["nc"] = _build()
    nc = _CACHE["nc"]

    in_maps = []
    for c in range(NCORES):
        sl = slice(c * IPC, (c + 1) * IPC)
        in_maps.append({
            "f1": np.ascontiguousarray(f1[sl]),
            "f2": np.ascontiguousarray(f2[sl]),
            "wt": wt, "b1": b1, "b2": b2, "t2x2": t2x2, "ident": ident,
        })

    res = bass_utils.run_bass_kernel_spmd(nc, in_maps,
                                          core_ids=list(range(NCORES)))
    out = np.empty((NI, C, HW), dtype=np.float32)
    for c in range(NCORES):
        out[c * IPC:(c + 1) * IPC] = res.results[c]["out"]
    return out.reshape(NI, C, H, W)
